# revision 22
# baseline (speedup 1.0000x reference)
"""Trainium2 Bass kernel for nn_Attn_Pred_Model (causal geometric-decay FIR + position biases).

Math:
  out[b,t,d] = alpha * sum_{i=0}^{P-1} beta^i * x[b,t-1-i,d]
               + pos_fwd[d] + pos_bwd[bucket(t,d)]

The FIR along the sequence dim is a banded (block-bidiagonal) Toeplitz matmul:
with 128-row sequence blocks,  y[blk] = D @ x[blk] + L @ x[blk-1]
for two constant 128x128 matrices D, L built from (alpha, beta) on the host.
The (S, 32) position bias is precomputed on the host and added on the
vector engine after the PE matmuls.

Sharding: pure data parallelism — batch dim split across the 8 NeuronCores.
The device-side layout is (S, B_loc, NB): the shard handed to each core is a
transposed *view*; the SPMD runner's input-concat materializes it (same
one-copy cost as contiguous sharding) and in exchange every DMA descriptor
is a 2-16KB contiguous run instead of 128B, which is the difference between
~170 GB/s and ~line-rate HBM bandwidth per core.
"""

import os
import sys

import numpy as np

os.environ.setdefault("MYCRO_LOCAL_CACHE", "1")
if "/opt/trn_rl_repo" not in sys.path:
    sys.path.insert(0, "/opt/trn_rl_repo")

B, S, NB = 1024, 1024, 32
NCORES = 8
B_LOC = B // NCORES  # batches per core
SB = 128             # sequence block size
NTB = S // SB        # sequence blocks
BC = 16              # batches per matmul chunk -> N = BC*NB = 512 columns
NCHUNK_FULL = B_LOC // BC
F32 = np.float32

_PROGRAM_CACHE = {}


def _install_ntff_shim():
    """Provide antenv.axon_hooks if the image lacks it, so trace=True works.

    The axon boot module ships a ctypes NTFF-profile hook but only registers
    it when ``antenv.axon_hooks`` exists; this image's antenv does not have
    that module, which makes ``run_bass_kernel_spmd(trace=True)`` crash on
    import. Inject an in-memory equivalent. No-op if tracing is never used.
    """
    try:
        import antenv.axon_hooks  # noqa: F401
        return
    except ImportError:
        pass
    try:
        import types

        import antenv
        from trn_agent_boot.trn_boot import _ntff_profile_via_ctypes

        hook = _ntff_profile_via_ctypes("/opt/axon/libaxon_pjrt.so")
        mod = types.ModuleType("antenv.axon_hooks")
        state = {"hook": hook}
        mod.get_axon_ntff_profile_hook = lambda: state["hook"]
        mod.set_axon_ntff_profile_hook = lambda h: state.__setitem__("hook", h)
        sys.modules["antenv.axon_hooks"] = mod
        antenv.axon_hooks = mod
    except Exception:
        pass


def _split_multi_waits(nc, maxw=1):
    """Work around a walrus limit in this image: instructions carrying more
    than ~2 sem waits die in codegen with "Too many sync wait commands".
    Move excess waits onto same-engine NoOps placed just before the
    instruction (identical sync semantics, negligible cost)."""
    import concourse.mybir as mybir

    for fn in nc.m.functions:
        for blk in fn.blocks:
            out = []
            changed = False
            for inst in blk.instructions:
                si = inst.sync_info
                if si is not None and len(si.on_wait) > maxw:
                    waits = list(si.on_wait)
                    excess, keep = waits[:-maxw], waits[-maxw:]
                    for k, w in enumerate(excess):
                        out.append(mybir.InstNoOp(
                            name=f"{inst.name}-sw{k}",
                            engine=inst.engine,
                            bass_nofuse=True,
                            sync_info=mybir.SyncInfo(on_wait=[w], on_update=[]),
                        ))
                    inst.sync_info = mybir.SyncInfo(
                        on_wait=list(keep), on_update=list(si.on_update))
                    changed = True
                out.append(inst)
            if changed:
                blk.instructions = out
    return nc


def build_program(b_loc=B_LOC, split_waits=True):
    """Per-core Bass/Tile program. Device-side x/out layout is (S, b_loc, NB).

    x and out travel as bf16 (host converts): halves HBM traffic vs fp32,
    which is the binding roofline. Matmuls run bf16 (1 col/cycle @ 2.4 GHz,
    fp32 PSUM accumulate). The PSUM->SBUF bias-add+cast pass alternates
    between the DVE and Pool engines so neither becomes a serial tail.

    split_waits=True post-processes for the HW compiler; pass False when the
    module is destined for CoreSim (the sim rejects the injected NoOps)."""
    import concourse.bass as bass
    import concourse.mybir as mybir
    import concourse.tile as tile

    f32 = mybir.dt.float32
    bf16 = mybir.dt.bfloat16
    nchunk = b_loc // BC

    nc = bass.Bass("TRN2")
    x_h = nc.declare_dram_parameter("x", [S, b_loc, NB], bf16, False)
    dt_h = nc.declare_dram_parameter("dmat", [SB, SB], bf16, False)   # D^T
    lt_h = nc.declare_dram_parameter("lmat", [SB, SB], bf16, False)   # L^T
    pb_h = nc.declare_dram_parameter("pbias", [NTB, SB, NB], f32, False)
    out_h = nc.declare_dram_parameter("out", [S, b_loc, NB], bf16, True)

    with tile.TileContext(nc) as tc:
        with (
            tc.tile_pool(name="consts", bufs=1) as cpool,
            tc.tile_pool(name="xin", bufs=NTB) as xpool,
            tc.tile_pool(name="outp", bufs=3) as opool,
            tc.tile_pool(name="tmp", bufs=6) as tpool,
            tc.tile_pool(name="psum", bufs=8, space="PSUM") as ppool,
        ):
            dt_sb = cpool.tile([SB, SB], bf16, tag="dt")
            lt_sb = cpool.tile([SB, SB], bf16, tag="lt")
            pb_sb = cpool.tile([SB, NTB, NB], f32, tag="pb")
            # small matmul consts ride ahead of x on the sync queue
            # (0.2us); pbias is only needed by the consumers, scalar queue
            nc.sync.dma_start(dt_sb[:], dt_h[:])
            nc.sync.dma_start(lt_sb[:], lt_h[:])
            nc.scalar.dma_start(pb_sb[:], pb_h[:].rearrange("t p d -> p t d"))

            hb = b_loc // 2  # half-block batch split for finer DMA/sync
            # All 8 x blocks are SBUF-resident; pre-issue every input DMA
            # up front, split across the two HARDWARE-DGE queues (SP and
            # ACT). One queue alone sustains ~274 GB/s; two run ~2x. The
            # gpsimd queue is software-DGE: bulk DMAs there stall the GP
            # engine itself (descriptor generation is inline), which
            # regressed two previous attempts. The input DMAs carry no
            # waits, so the output DMAs issued later on the same queues
            # drain right behind them without head-of-line blocking.
            xts = [xpool.tile([SB, b_loc, NB], bf16, tag="xt",
                              name=f"xt{i}") for i in range(NTB)]
            for tb in range(NTB):
                r = slice(tb * SB, (tb + 1) * SB)
                eng = nc.sync if tb % 2 == 0 else nc.scalar
                if tb == 0:
                    # split the first block so its first half lands (and
                    # the D-phase starts) ~2us sooner
                    eng.dma_start(xts[0][:, :hb, :], x_h[r, :hb, :])
                    eng.dma_start(xts[0][:, hb:, :], x_h[r, hb:, :])
                else:
                    eng.dma_start(xts[tb][:], x_h[r])

            prev_xt = None
            for tb in range(NTB):
                xt = xts[tb]
                r = slice(tb * SB, (tb + 1) * SB)
                ot = opool.tile([SB, b_loc, NB], bf16, tag="ot")
                bias = pb_sb[:, tb:tb + 1, :].broadcast_to((SB, BC, NB))
                # D phase then L phase (fewer stationary-weight switches);
                # consumers drain each half so its output DMA fires early
                for half in range(2):
                    cs = range(half * nchunk // 2, (half + 1) * nchunk // 2)
                    pss = {}
                    for c in cs:
                        bs = slice(c * BC, (c + 1) * BC)
                        ps = ppool.tile([SB, BC, NB], f32, tag="ps")
                        nc.tensor.matmul(ps[:], dt_sb[:], xt[:, bs, :],
                                         start=True, stop=(tb == 0))
                        pss[c] = ps
                    if tb > 0:
                        for c in cs:
                            bs = slice(c * BC, (c + 1) * BC)
                            nc.tensor.matmul(pss[c][:], lt_sb[:],
                                             prev_xt[:, bs, :],
                                             start=False, stop=True)
                    # PSUM -> SBUF bias-add + bf16 cast, split across
                    # engines: DVE handles most chunks directly (it can
                    # read PSUM); ACT copies the rest to a temp and GPSIMD
                    # (no PSUM access on TRN2) adds the bias from there.
                    # The slow ACT+GP path takes the half's FIRST chunks so
                    # a fast DVE chunk is what gates the output DMA.
                    for c in cs:
                        bs = slice(c * BC, (c + 1) * BC)
                        if c in (0, 1, 4):
                            tmp = tpool.tile([SB, BC, NB], bf16, tag="tmp")
                            nc.scalar.copy(tmp[:], pss[c][:])
                            nc.gpsimd.tensor_tensor(ot[:, bs, :], tmp[:],
                                                    bias,
                                                    mybir.AluOpType.add)
                    for c in cs:
                        bs = slice(c * BC, (c + 1) * BC)
                        if c not in (0, 1, 4):
                            nc.vector.tensor_tensor(ot[:, bs, :], pss[c][:],
                                                    bias, mybir.AluOpType.add)
                    hs = slice(half * hb, (half + 1) * hb)
                    # output halves split across the scalar and sync
                    # queues; sync's queue is clear of input work by the
                    # time output volume ramps
                    oeng = nc.scalar if half == 0 else nc.sync
                    oeng.dma_start(out_h[r, hs, :], ot[:, hs, :])
                prev_xt = xt
    return _split_multi_waits(nc) if split_waits else nc


def to_bf16(a):
    """Convert to bfloat16 (ml_dtypes) for the device-side bf16 datapath."""
    import ml_dtypes

    return np.ascontiguousarray(np.asarray(a, dtype=F32)).astype(
        ml_dtypes.bfloat16)


def host_consts(alpha, beta, pos_fwd_param, pos_bwd_param, past_steps):
    """Precompute D^T, L^T (128x128 FIR block matrices) and the position bias."""
    P = int(np.asarray(past_steps).reshape(-1)[0]) if np.ndim(past_steps) else int(past_steps)
    assert P <= SB, f"past_steps {P} > block size {SB} unsupported"
    a = float(np.asarray(alpha).reshape(-1)[0])
    b = float(np.asarray(beta).reshape(-1)[0])
    w = a * np.power(b, np.arange(P, dtype=np.float64))

    idx = np.arange(SB)
    km = idx[:, None] - idx[None, :]          # t - s
    D = np.where((km >= 1) & (km <= P), w[np.clip(km - 1, 0, P - 1)], 0.0)
    kml = km + SB                             # cross-block: t - s + 128
    L = np.where((kml >= 1) & (kml <= P), w[np.clip(kml - 1, 0, P - 1)], 0.0)
    DT = to_bf16(D.T)
    LT = to_bf16(L.T)

    t = np.arange(S)[:, None]
    j = np.arange(NB)[None, :]
    bucket = ((t - NB * j) % S) // NB         # (S, NB)
    pf = np.asarray(pos_fwd_param, dtype=np.float64).reshape(NB)
    pbw = np.asarray(pos_bwd_param, dtype=np.float64).reshape(NB)
    pb = pf[None, :] + pbw[bucket]            # (S, NB)
    pbias = np.ascontiguousarray(pb.reshape(NTB, SB, NB), dtype=F32)
    return DT, LT, pbias


def reference_numpy(x, alpha, beta, pos_fwd_param, pos_bwd_param, past_steps):
    """Float64 host reference (for self-tests)."""
    P = int(past_steps)
    a = float(np.asarray(alpha).reshape(-1)[0])
    b = float(np.asarray(beta).reshape(-1)[0])
    w = a * np.power(b, np.arange(P, dtype=np.float64))
    xf = np.asarray(x, dtype=np.float64)
    Bn, Sn, Dn = xf.shape
    y = np.zeros_like(xf)
    for i in range(P):
        y[:, i + 1:, :] += w[i] * xf[:, :Sn - 1 - i, :]
    t = np.arange(Sn)[:, None]
    j = np.arange(Dn)[None, :]
    bucket = ((t - Dn * j) % Sn) // Dn
    pf = np.asarray(pos_fwd_param, dtype=np.float64).reshape(Dn)
    pbw = np.asarray(pos_bwd_param, dtype=np.float64).reshape(Dn)
    return y + pf[None, :] + pbw[bucket]


def kernel(x, alpha, beta, pos_fwd_param, pos_bwd_param, past_steps):
    _install_ntff_shim()
    from concourse.bass_utils import run_bass_kernel_spmd

    x = np.asarray(x)
    assert x.shape == (B, S, NB), x.shape
    x = to_bf16(x)  # device datapath is bf16; halves HBM traffic
    DT, LT, pbias = host_consts(alpha, beta, pos_fwd_param, pos_bwd_param,
                                past_steps)

    if "hw" not in _PROGRAM_CACHE:
        _PROGRAM_CACHE["hw"] = build_program(B_LOC)
    nc = _PROGRAM_CACHE["hw"]

    core_ids = list(range(NCORES))
    in_maps = [
        {
            # transposed view (S, B_LOC, NB); materialized by the runner's
            # input concat — no extra host copy vs contiguous sharding
            "x": x[i * B_LOC:(i + 1) * B_LOC].transpose(1, 0, 2),
            "dmat": DT,
            "lmat": LT,
            "pbias": pbias,
        }
        for i in core_ids
    ]
    res = run_bass_kernel_spmd(nc, in_maps, core_ids)
    out = np.empty((B, S, NB), dtype=F32)
    for i in core_ids:
        out[i * B_LOC:(i + 1) * B_LOC] = (
            res.results[i]["out"].astype(F32).transpose(1, 0, 2))
    if res.exec_time_ns is not None:
        kernel.last_exec_time_ns = res.exec_time_ns
    kernel.last_results = res
    return out


kernel.last_exec_time_ns = None
kernel.last_results = None



# revision 24
# speedup vs baseline: 1.0657x; 1.0657x over previous
"""Trainium2 Bass kernel for nn_Attn_Pred_Model (causal geometric-decay FIR + position biases).

Math:
  out[b,t,d] = alpha * sum_{i=0}^{P-1} beta^i * x[b,t-1-i,d]
               + pos_fwd[d] + pos_bwd[bucket(t,d)]

The FIR along the sequence dim is a banded (block-bidiagonal) Toeplitz matmul:
with 128-row sequence blocks,  y[blk] = D @ x[blk] + L @ x[blk-1]
for two constant 128x128 matrices D, L built from (alpha, beta) on the host.
The (S, 32) position bias is precomputed on the host and added on the
vector engine after the PE matmuls.

Sharding: pure data parallelism — batch dim split across the 8 NeuronCores.
The device-side layout is (S, B_loc, NB): the shard handed to each core is a
transposed *view*; the SPMD runner's input-concat materializes it (same
one-copy cost as contiguous sharding) and in exchange every DMA descriptor
is a 2-16KB contiguous run instead of 128B, which is the difference between
~170 GB/s and ~line-rate HBM bandwidth per core.
"""

import os
import sys

import numpy as np

os.environ.setdefault("MYCRO_LOCAL_CACHE", "1")
if "/opt/trn_rl_repo" not in sys.path:
    sys.path.insert(0, "/opt/trn_rl_repo")

B, S, NB = 1024, 1024, 32
NCORES = 8
B_LOC = B // NCORES  # batches per core
SB = 128             # sequence block size
NTB = S // SB        # sequence blocks
BC = 16              # batches per matmul chunk -> N = BC*NB = 512 columns
NCHUNK_FULL = B_LOC // BC
F32 = np.float32

_PROGRAM_CACHE = {}


def _install_ntff_shim():
    """Provide antenv.axon_hooks if the image lacks it, so trace=True works.

    The axon boot module ships a ctypes NTFF-profile hook but only registers
    it when ``antenv.axon_hooks`` exists; this image's antenv does not have
    that module, which makes ``run_bass_kernel_spmd(trace=True)`` crash on
    import. Inject an in-memory equivalent. No-op if tracing is never used.
    """
    try:
        import antenv.axon_hooks  # noqa: F401
        return
    except ImportError:
        pass
    try:
        import types

        import antenv
        from trn_agent_boot.trn_boot import _ntff_profile_via_ctypes

        hook = _ntff_profile_via_ctypes("/opt/axon/libaxon_pjrt.so")
        mod = types.ModuleType("antenv.axon_hooks")
        state = {"hook": hook}
        mod.get_axon_ntff_profile_hook = lambda: state["hook"]
        mod.set_axon_ntff_profile_hook = lambda h: state.__setitem__("hook", h)
        sys.modules["antenv.axon_hooks"] = mod
        antenv.axon_hooks = mod
    except Exception:
        pass


def _split_multi_waits(nc, maxw=1):
    """Work around a walrus limit in this image: instructions carrying more
    than ~2 sem waits die in codegen with "Too many sync wait commands".
    Move excess waits onto same-engine NoOps placed just before the
    instruction (identical sync semantics, negligible cost)."""
    import concourse.mybir as mybir

    for fn in nc.m.functions:
        for blk in fn.blocks:
            out = []
            changed = False
            for inst in blk.instructions:
                si = inst.sync_info
                if si is not None and len(si.on_wait) > maxw:
                    waits = list(si.on_wait)
                    excess, keep = waits[:-maxw], waits[-maxw:]
                    for k, w in enumerate(excess):
                        out.append(mybir.InstNoOp(
                            name=f"{inst.name}-sw{k}",
                            engine=inst.engine,
                            bass_nofuse=True,
                            sync_info=mybir.SyncInfo(on_wait=[w], on_update=[]),
                        ))
                    inst.sync_info = mybir.SyncInfo(
                        on_wait=list(keep), on_update=list(si.on_update))
                    changed = True
                out.append(inst)
            if changed:
                blk.instructions = out
    return nc


def build_program(b_loc=B_LOC, split_waits=True):
    """Per-core Bass/Tile program. Device-side x/out layout is (S, b_loc, NB).

    x and out travel as bf16 (host converts): halves HBM traffic vs fp32,
    which is the binding roofline. Matmuls run bf16 (1 col/cycle @ 2.4 GHz,
    fp32 PSUM accumulate). The PSUM->SBUF bias-add+cast pass alternates
    between the DVE and Pool engines so neither becomes a serial tail.

    split_waits=True post-processes for the HW compiler; pass False when the
    module is destined for CoreSim (the sim rejects the injected NoOps)."""
    import concourse.bass as bass
    import concourse.mybir as mybir
    import concourse.tile as tile

    f32 = mybir.dt.float32
    bf16 = mybir.dt.bfloat16
    nchunk = b_loc // BC

    nc = bass.Bass("TRN2")
    x_h = nc.declare_dram_parameter("x", [S, b_loc, NB], bf16, False)
    dt_h = nc.declare_dram_parameter("dmat", [SB, SB], bf16, False)   # D^T
    lt_h = nc.declare_dram_parameter("lmat", [SB, SB], bf16, False)   # L^T
    pb_h = nc.declare_dram_parameter("pbias", [NTB, SB, NB], f32, False)
    out_h = nc.declare_dram_parameter("out", [S, b_loc, NB], bf16, True)

    with tile.TileContext(nc) as tc:
        with (
            tc.tile_pool(name="consts", bufs=1) as cpool,
            tc.tile_pool(name="xin", bufs=NTB) as xpool,
            tc.tile_pool(name="outp", bufs=3) as opool,
            tc.tile_pool(name="tmp", bufs=6) as tpool,
            tc.tile_pool(name="psum", bufs=8, space="PSUM") as ppool,
        ):
            dt_sb = cpool.tile([SB, SB], bf16, tag="dt")
            lt_sb = cpool.tile([SB, SB], bf16, tag="lt")
            pb_sb = cpool.tile([SB, NTB, NB], f32, tag="pb")
            # dt/lt lead the sync queue (0.2us, needed by the first
            # matmul); pbias leads the then-idle scalar queue so the
            # first consumer TTs aren't gated on it (on the slow gpsimd
            # queue it landed at ~17us and stalled the whole conveyor)
            nc.sync.dma_start(dt_sb[:], dt_h[:])
            nc.sync.dma_start(lt_sb[:], lt_h[:])
            nc.scalar.dma_start(pb_sb[:], pb_h[:].rearrange("t p d -> p t d"))

            hb = b_loc // 2  # half-block batch split for finer DMA/sync
            # Bulk input stays on the sync queue alone: a DMA ring only
            # holds ~2 one-MB transfers' descriptors, and the issuing
            # engine STALLS on the third — on scalar/gpsimd that freezes
            # the ACT/GP compute work behind it (measured twice). SP has
            # no other duties, so only it may carry deep bulk traffic.
            xts = [xpool.tile([SB, b_loc, NB], bf16, tag="xt",
                              name=f"xt{i}") for i in range(NTB)]
            for tb in range(NTB):
                r = slice(tb * SB, (tb + 1) * SB)
                if tb in (0, NTB - 1):
                    # split first and last blocks: the first so compute
                    # starts ~2us sooner, the last so its D-phase starts
                    # before the full block lands
                    nc.sync.dma_start(xts[tb][:, :hb, :], x_h[r, :hb, :])
                    nc.sync.dma_start(xts[tb][:, hb:, :], x_h[r, hb:, :])
                else:
                    nc.sync.dma_start(xts[tb][:], x_h[r])

            prev_xt = None
            for tb in range(NTB):
                xt = xts[tb]
                r = slice(tb * SB, (tb + 1) * SB)
                ot = opool.tile([SB, b_loc, NB], bf16, tag="ot")
                bias = pb_sb[:, tb:tb + 1, :].broadcast_to((SB, BC, NB))
                # D phase then L phase (fewer stationary-weight switches);
                # consumers drain each half so its output DMA fires early
                for half in range(2):
                    cs = range(half * nchunk // 2, (half + 1) * nchunk // 2)
                    pss = {}
                    for c in cs:
                        bs = slice(c * BC, (c + 1) * BC)
                        ps = ppool.tile([SB, BC, NB], f32, tag="ps")
                        nc.tensor.matmul(ps[:], dt_sb[:], xt[:, bs, :],
                                         start=True, stop=(tb == 0))
                        pss[c] = ps
                    if tb > 0:
                        for c in cs:
                            bs = slice(c * BC, (c + 1) * BC)
                            nc.tensor.matmul(pss[c][:], lt_sb[:],
                                             prev_xt[:, bs, :],
                                             start=False, stop=True)
                    # PSUM -> SBUF bias-add + bf16 cast, split across
                    # engines: DVE handles most chunks directly (it can
                    # read PSUM); ACT copies the rest to a temp and GPSIMD
                    # (no PSUM access on TRN2) adds the bias from there.
                    # The slow ACT+GP path takes the half's FIRST chunks so
                    # a fast DVE chunk is what gates the output DMA.
                    # the last block's conveyor gates the kernel tail:
                    # keep slow GP off it entirely there
                    gp_chunks = () if tb == NTB - 1 else (0, 1, 4)
                    for c in cs:
                        bs = slice(c * BC, (c + 1) * BC)
                        if c in gp_chunks:
                            tmp = tpool.tile([SB, BC, NB], bf16, tag="tmp")
                            nc.scalar.copy(tmp[:], pss[c][:])
                            nc.gpsimd.tensor_tensor(ot[:, bs, :], tmp[:],
                                                    bias,
                                                    mybir.AluOpType.add)
                    for c in cs:
                        bs = slice(c * BC, (c + 1) * BC)
                        if c not in gp_chunks:
                            nc.vector.tensor_tensor(ot[:, bs, :], pss[c][:],
                                                    bias, mybir.AluOpType.add)
                    hs = slice(half * hb, (half + 1) * hb)
                    # outputs ride the scalar queue; for the last two
                    # blocks the h1 halves use sync's queue (idle once
                    # the input stream drains) so the final halves land
                    # in parallel
                    oeng = (nc.sync if (half == 1 and tb >= NTB - 2)
                            else nc.scalar)
                    oeng.dma_start(out_h[r, hs, :], ot[:, hs, :])
                prev_xt = xt
    return _split_multi_waits(nc) if split_waits else nc


def to_bf16(a):
    """Convert to bfloat16 (ml_dtypes) for the device-side bf16 datapath."""
    import ml_dtypes

    return np.ascontiguousarray(np.asarray(a, dtype=F32)).astype(
        ml_dtypes.bfloat16)


def host_consts(alpha, beta, pos_fwd_param, pos_bwd_param, past_steps):
    """Precompute D^T, L^T (128x128 FIR block matrices) and the position bias."""
    P = int(np.asarray(past_steps).reshape(-1)[0]) if np.ndim(past_steps) else int(past_steps)
    assert P <= SB, f"past_steps {P} > block size {SB} unsupported"
    a = float(np.asarray(alpha).reshape(-1)[0])
    b = float(np.asarray(beta).reshape(-1)[0])
    w = a * np.power(b, np.arange(P, dtype=np.float64))

    idx = np.arange(SB)
    km = idx[:, None] - idx[None, :]          # t - s
    D = np.where((km >= 1) & (km <= P), w[np.clip(km - 1, 0, P - 1)], 0.0)
    kml = km + SB                             # cross-block: t - s + 128
    L = np.where((kml >= 1) & (kml <= P), w[np.clip(kml - 1, 0, P - 1)], 0.0)
    DT = to_bf16(D.T)
    LT = to_bf16(L.T)

    t = np.arange(S)[:, None]
    j = np.arange(NB)[None, :]
    bucket = ((t - NB * j) % S) // NB         # (S, NB)
    pf = np.asarray(pos_fwd_param, dtype=np.float64).reshape(NB)
    pbw = np.asarray(pos_bwd_param, dtype=np.float64).reshape(NB)
    pb = pf[None, :] + pbw[bucket]            # (S, NB)
    pbias = np.ascontiguousarray(pb.reshape(NTB, SB, NB), dtype=F32)
    return DT, LT, pbias


def reference_numpy(x, alpha, beta, pos_fwd_param, pos_bwd_param, past_steps):
    """Float64 host reference (for self-tests)."""
    P = int(past_steps)
    a = float(np.asarray(alpha).reshape(-1)[0])
    b = float(np.asarray(beta).reshape(-1)[0])
    w = a * np.power(b, np.arange(P, dtype=np.float64))
    xf = np.asarray(x, dtype=np.float64)
    Bn, Sn, Dn = xf.shape
    y = np.zeros_like(xf)
    for i in range(P):
        y[:, i + 1:, :] += w[i] * xf[:, :Sn - 1 - i, :]
    t = np.arange(Sn)[:, None]
    j = np.arange(Dn)[None, :]
    bucket = ((t - Dn * j) % Sn) // Dn
    pf = np.asarray(pos_fwd_param, dtype=np.float64).reshape(Dn)
    pbw = np.asarray(pos_bwd_param, dtype=np.float64).reshape(Dn)
    return y + pf[None, :] + pbw[bucket]


def kernel(x, alpha, beta, pos_fwd_param, pos_bwd_param, past_steps):
    _install_ntff_shim()
    from concourse.bass_utils import run_bass_kernel_spmd

    x = np.asarray(x)
    assert x.shape == (B, S, NB), x.shape
    x = to_bf16(x)  # device datapath is bf16; halves HBM traffic
    DT, LT, pbias = host_consts(alpha, beta, pos_fwd_param, pos_bwd_param,
                                past_steps)

    if "hw" not in _PROGRAM_CACHE:
        _PROGRAM_CACHE["hw"] = build_program(B_LOC)
    nc = _PROGRAM_CACHE["hw"]

    core_ids = list(range(NCORES))
    in_maps = [
        {
            # transposed view (S, B_LOC, NB); materialized by the runner's
            # input concat — no extra host copy vs contiguous sharding
            "x": x[i * B_LOC:(i + 1) * B_LOC].transpose(1, 0, 2),
            "dmat": DT,
            "lmat": LT,
            "pbias": pbias,
        }
        for i in core_ids
    ]
    res = run_bass_kernel_spmd(nc, in_maps, core_ids)
    out = np.empty((B, S, NB), dtype=F32)
    for i in core_ids:
        out[i * B_LOC:(i + 1) * B_LOC] = (
            res.results[i]["out"].astype(F32).transpose(1, 0, 2))
    if res.exec_time_ns is not None:
        kernel.last_exec_time_ns = res.exec_time_ns
    kernel.last_results = res
    return out


kernel.last_exec_time_ns = None
kernel.last_results = None



# revision 26
# speedup vs baseline: 1.0683x; 1.0025x over previous
"""Trainium2 Bass kernel for nn_Attn_Pred_Model (causal geometric-decay FIR + position biases).

Math:
  out[b,t,d] = alpha * sum_{i=0}^{P-1} beta^i * x[b,t-1-i,d]
               + pos_fwd[d] + pos_bwd[bucket(t,d)]

The FIR along the sequence dim is a banded (block-bidiagonal) Toeplitz matmul:
with 128-row sequence blocks,  y[blk] = D @ x[blk] + L @ x[blk-1]
for two constant 128x128 matrices D, L built from (alpha, beta) on the host.
The (S, 32) position bias is precomputed on the host and added on the
vector engine after the PE matmuls.

Sharding: pure data parallelism — batch dim split across the 8 NeuronCores.
The device-side layout is (S, B_loc, NB): the shard handed to each core is a
transposed *view*; the SPMD runner's input-concat materializes it (same
one-copy cost as contiguous sharding) and in exchange every DMA descriptor
is a 2-16KB contiguous run instead of 128B, which is the difference between
~170 GB/s and ~line-rate HBM bandwidth per core.
"""

import os
import sys

import numpy as np

os.environ.setdefault("MYCRO_LOCAL_CACHE", "1")
if "/opt/trn_rl_repo" not in sys.path:
    sys.path.insert(0, "/opt/trn_rl_repo")

B, S, NB = 1024, 1024, 32
NCORES = 8
B_LOC = B // NCORES  # batches per core
SB = 128             # sequence block size
NTB = S // SB        # sequence blocks
BC = 16              # batches per matmul chunk -> N = BC*NB = 512 columns
NCHUNK_FULL = B_LOC // BC
F32 = np.float32

_PROGRAM_CACHE = {}


def _install_ntff_shim():
    """Provide antenv.axon_hooks if the image lacks it, so trace=True works.

    The axon boot module ships a ctypes NTFF-profile hook but only registers
    it when ``antenv.axon_hooks`` exists; this image's antenv does not have
    that module, which makes ``run_bass_kernel_spmd(trace=True)`` crash on
    import. Inject an in-memory equivalent. No-op if tracing is never used.
    """
    try:
        import antenv.axon_hooks  # noqa: F401
        return
    except ImportError:
        pass
    try:
        import types

        import antenv
        from trn_agent_boot.trn_boot import _ntff_profile_via_ctypes

        hook = _ntff_profile_via_ctypes("/opt/axon/libaxon_pjrt.so")
        mod = types.ModuleType("antenv.axon_hooks")
        state = {"hook": hook}
        mod.get_axon_ntff_profile_hook = lambda: state["hook"]
        mod.set_axon_ntff_profile_hook = lambda h: state.__setitem__("hook", h)
        sys.modules["antenv.axon_hooks"] = mod
        antenv.axon_hooks = mod
    except Exception:
        pass


def _split_multi_waits(nc, maxw=1):
    """Work around a walrus limit in this image: instructions carrying more
    than ~2 sem waits die in codegen with "Too many sync wait commands".
    Move excess waits onto same-engine NoOps placed just before the
    instruction (identical sync semantics, negligible cost)."""
    import concourse.mybir as mybir

    for fn in nc.m.functions:
        for blk in fn.blocks:
            out = []
            changed = False
            for inst in blk.instructions:
                si = inst.sync_info
                if si is not None and len(si.on_wait) > maxw:
                    waits = list(si.on_wait)
                    excess, keep = waits[:-maxw], waits[-maxw:]
                    for k, w in enumerate(excess):
                        out.append(mybir.InstNoOp(
                            name=f"{inst.name}-sw{k}",
                            engine=inst.engine,
                            bass_nofuse=True,
                            sync_info=mybir.SyncInfo(on_wait=[w], on_update=[]),
                        ))
                    inst.sync_info = mybir.SyncInfo(
                        on_wait=list(keep), on_update=list(si.on_update))
                    changed = True
                out.append(inst)
            if changed:
                blk.instructions = out
    return nc


def build_program(b_loc=B_LOC, split_waits=True):
    """Per-core Bass/Tile program. Device-side x/out layout is (S, b_loc, NB).

    x and out travel as bf16 (host converts): halves HBM traffic vs fp32,
    which is the binding roofline. Matmuls run bf16 (1 col/cycle @ 2.4 GHz,
    fp32 PSUM accumulate). The PSUM->SBUF bias-add+cast pass alternates
    between the DVE and Pool engines so neither becomes a serial tail.

    split_waits=True post-processes for the HW compiler; pass False when the
    module is destined for CoreSim (the sim rejects the injected NoOps)."""
    import concourse.bass as bass
    import concourse.mybir as mybir
    import concourse.tile as tile

    f32 = mybir.dt.float32
    bf16 = mybir.dt.bfloat16
    nchunk = b_loc // BC

    nc = bass.Bass("TRN2")
    x_h = nc.declare_dram_parameter("x", [S, b_loc, NB], bf16, False)
    dt_h = nc.declare_dram_parameter("dmat", [SB, SB], bf16, False)   # D^T
    lt_h = nc.declare_dram_parameter("lmat", [SB, SB], bf16, False)   # L^T
    pb_h = nc.declare_dram_parameter("pbias", [NTB, SB, NB], f32, False)
    out_h = nc.declare_dram_parameter("out", [S, b_loc, NB], bf16, True)

    with tile.TileContext(nc) as tc:
        with (
            tc.tile_pool(name="consts", bufs=1) as cpool,
            tc.tile_pool(name="xin", bufs=6) as xpool,
            tc.tile_pool(name="outp", bufs=3) as opool,
            tc.tile_pool(name="tmp", bufs=6) as tpool,
            tc.tile_pool(name="psum", bufs=8, space="PSUM") as ppool,
        ):
            dt_sb = cpool.tile([SB, SB], bf16, tag="dt")
            lt_sb = cpool.tile([SB, SB], bf16, tag="lt")
            pb_sb = cpool.tile([SB, NTB, NB], f32, tag="pb")
            # dt/lt lead the sync queue (0.2us, needed by the first
            # matmul); pbias leads the then-idle scalar queue so the
            # first consumer TTs aren't gated on it (on the slow gpsimd
            # queue it landed at ~17us and stalled the whole conveyor)
            nc.sync.dma_start(dt_sb[:], dt_h[:])
            nc.sync.dma_start(lt_sb[:], lt_h[:])
            nc.scalar.dma_start(pb_sb[:], pb_h[:].rearrange("t p d -> p t d"))

            hb = b_loc // 2  # half-block batch split for finer DMA/sync
            # Bulk input stays on the sync queue alone, issued in-loop:
            # pre-issuing all blocks up front measurably SLOWS the queue
            # (274 -> ~180 GB/s, three attempts), and bulk DMAs issued
            # from scalar/gpsimd stall those engines' compute work
            # behind the issue (ring backpressure). SP issuing one block
            # per iteration with a deep-enough pool is the fast mode.
            prev_xt = None
            for tb in range(NTB):
                xt = xpool.tile([SB, b_loc, NB], bf16, tag="xt")
                r = slice(tb * SB, (tb + 1) * SB)
                if tb in (0, NTB - 1):
                    # split first and last blocks: the first so compute
                    # starts ~2us sooner, the last so its D-phase starts
                    # before the full block lands
                    nc.sync.dma_start(xt[:, :hb, :], x_h[r, :hb, :])
                    nc.sync.dma_start(xt[:, hb:, :], x_h[r, hb:, :])
                else:
                    nc.sync.dma_start(xt[:], x_h[r])
                ot = opool.tile([SB, b_loc, NB], bf16, tag="ot")
                bias = pb_sb[:, tb:tb + 1, :].broadcast_to((SB, BC, NB))
                # D phase then L phase (fewer stationary-weight switches);
                # consumers drain each half so its output DMA fires early
                for half in range(2):
                    cs = range(half * nchunk // 2, (half + 1) * nchunk // 2)
                    pss = {}
                    for c in cs:
                        bs = slice(c * BC, (c + 1) * BC)
                        ps = ppool.tile([SB, BC, NB], f32, tag="ps")
                        nc.tensor.matmul(ps[:], dt_sb[:], xt[:, bs, :],
                                         start=True, stop=(tb == 0))
                        pss[c] = ps
                    if tb > 0:
                        for c in cs:
                            bs = slice(c * BC, (c + 1) * BC)
                            nc.tensor.matmul(pss[c][:], lt_sb[:],
                                             prev_xt[:, bs, :],
                                             start=False, stop=True)
                    # PSUM -> SBUF bias-add + bf16 cast, split across
                    # engines: DVE handles most chunks directly (it can
                    # read PSUM); ACT copies the rest to a temp and GPSIMD
                    # (no PSUM access on TRN2) adds the bias from there.
                    # The slow ACT+GP path takes the half's FIRST chunks so
                    # a fast DVE chunk is what gates the output DMA.
                    # the last block's conveyor gates the kernel tail:
                    # keep slow GP off it entirely there
                    gp_chunks = () if tb == NTB - 1 else (0, 1, 4)
                    for c in cs:
                        bs = slice(c * BC, (c + 1) * BC)
                        if c in gp_chunks:
                            tmp = tpool.tile([SB, BC, NB], bf16, tag="tmp")
                            nc.scalar.copy(tmp[:], pss[c][:])
                            nc.gpsimd.tensor_tensor(ot[:, bs, :], tmp[:],
                                                    bias,
                                                    mybir.AluOpType.add)
                    for c in cs:
                        bs = slice(c * BC, (c + 1) * BC)
                        if c not in gp_chunks:
                            nc.vector.tensor_tensor(ot[:, bs, :], pss[c][:],
                                                    bias, mybir.AluOpType.add)
                    hs = slice(half * hb, (half + 1) * hb)
                    # outputs ride the scalar queue; for the last two
                    # blocks the h1 halves use sync's queue (idle once
                    # the input stream drains) so the final halves land
                    # in parallel
                    oeng = (nc.sync if (half == 1 and tb >= NTB - 2)
                            else nc.scalar)
                    oeng.dma_start(out_h[r, hs, :], ot[:, hs, :])
                prev_xt = xt
    return _split_multi_waits(nc) if split_waits else nc


def to_bf16(a):
    """Convert to bfloat16 (ml_dtypes) for the device-side bf16 datapath."""
    import ml_dtypes

    return np.ascontiguousarray(np.asarray(a, dtype=F32)).astype(
        ml_dtypes.bfloat16)


def host_consts(alpha, beta, pos_fwd_param, pos_bwd_param, past_steps):
    """Precompute D^T, L^T (128x128 FIR block matrices) and the position bias."""
    P = int(np.asarray(past_steps).reshape(-1)[0]) if np.ndim(past_steps) else int(past_steps)
    assert P <= SB, f"past_steps {P} > block size {SB} unsupported"
    a = float(np.asarray(alpha).reshape(-1)[0])
    b = float(np.asarray(beta).reshape(-1)[0])
    w = a * np.power(b, np.arange(P, dtype=np.float64))

    idx = np.arange(SB)
    km = idx[:, None] - idx[None, :]          # t - s
    D = np.where((km >= 1) & (km <= P), w[np.clip(km - 1, 0, P - 1)], 0.0)
    kml = km + SB                             # cross-block: t - s + 128
    L = np.where((kml >= 1) & (kml <= P), w[np.clip(kml - 1, 0, P - 1)], 0.0)
    DT = to_bf16(D.T)
    LT = to_bf16(L.T)

    t = np.arange(S)[:, None]
    j = np.arange(NB)[None, :]
    bucket = ((t - NB * j) % S) // NB         # (S, NB)
    pf = np.asarray(pos_fwd_param, dtype=np.float64).reshape(NB)
    pbw = np.asarray(pos_bwd_param, dtype=np.float64).reshape(NB)
    pb = pf[None, :] + pbw[bucket]            # (S, NB)
    pbias = np.ascontiguousarray(pb.reshape(NTB, SB, NB), dtype=F32)
    return DT, LT, pbias


def reference_numpy(x, alpha, beta, pos_fwd_param, pos_bwd_param, past_steps):
    """Float64 host reference (for self-tests)."""
    P = int(past_steps)
    a = float(np.asarray(alpha).reshape(-1)[0])
    b = float(np.asarray(beta).reshape(-1)[0])
    w = a * np.power(b, np.arange(P, dtype=np.float64))
    xf = np.asarray(x, dtype=np.float64)
    Bn, Sn, Dn = xf.shape
    y = np.zeros_like(xf)
    for i in range(P):
        y[:, i + 1:, :] += w[i] * xf[:, :Sn - 1 - i, :]
    t = np.arange(Sn)[:, None]
    j = np.arange(Dn)[None, :]
    bucket = ((t - Dn * j) % Sn) // Dn
    pf = np.asarray(pos_fwd_param, dtype=np.float64).reshape(Dn)
    pbw = np.asarray(pos_bwd_param, dtype=np.float64).reshape(Dn)
    return y + pf[None, :] + pbw[bucket]


def kernel(x, alpha, beta, pos_fwd_param, pos_bwd_param, past_steps):
    _install_ntff_shim()
    from concourse.bass_utils import run_bass_kernel_spmd

    x = np.asarray(x)
    assert x.shape == (B, S, NB), x.shape
    x = to_bf16(x)  # device datapath is bf16; halves HBM traffic
    DT, LT, pbias = host_consts(alpha, beta, pos_fwd_param, pos_bwd_param,
                                past_steps)

    if "hw" not in _PROGRAM_CACHE:
        _PROGRAM_CACHE["hw"] = build_program(B_LOC)
    nc = _PROGRAM_CACHE["hw"]

    core_ids = list(range(NCORES))
    in_maps = [
        {
            # transposed view (S, B_LOC, NB); materialized by the runner's
            # input concat — no extra host copy vs contiguous sharding
            "x": x[i * B_LOC:(i + 1) * B_LOC].transpose(1, 0, 2),
            "dmat": DT,
            "lmat": LT,
            "pbias": pbias,
        }
        for i in core_ids
    ]
    res = run_bass_kernel_spmd(nc, in_maps, core_ids)
    out = np.empty((B, S, NB), dtype=F32)
    for i in core_ids:
        out[i * B_LOC:(i + 1) * B_LOC] = (
            res.results[i]["out"].astype(F32).transpose(1, 0, 2))
    if res.exec_time_ns is not None:
        kernel.last_exec_time_ns = res.exec_time_ns
    kernel.last_results = res
    return out


kernel.last_exec_time_ns = None
kernel.last_results = None



# revision 32
# speedup vs baseline: 1.1183x; 1.0467x over previous
"""Trainium2 Bass kernel for nn_Attn_Pred_Model (causal geometric-decay FIR + position biases).

Math:
  out[b,t,d] = alpha * sum_{i=0}^{P-1} beta^i * x[b,t-1-i,d]
               + pos_fwd[d] + pos_bwd[bucket(t,d)]

The FIR along the sequence dim is a banded (block-bidiagonal) Toeplitz matmul:
with 128-row sequence blocks,  y[blk] = D @ x[blk] + L @ x[blk-1]
for two constant 128x128 matrices D, L built from (alpha, beta) on the host.
The (S, 32) position bias is precomputed on the host and added on the
vector engine after the PE matmuls.

Sharding: pure data parallelism — batch dim split across the 8 NeuronCores.
The device-side layout is (S, B_loc, NB): the shard handed to each core is a
transposed *view*; the SPMD runner's input-concat materializes it (same
one-copy cost as contiguous sharding) and in exchange every DMA descriptor
is a 2-16KB contiguous run instead of 128B, which is the difference between
~170 GB/s and ~line-rate HBM bandwidth per core.
"""

import os
import sys

import numpy as np

os.environ.setdefault("MYCRO_LOCAL_CACHE", "1")
if "/opt/trn_rl_repo" not in sys.path:
    sys.path.insert(0, "/opt/trn_rl_repo")

B, S, NB = 1024, 1024, 32
NCORES = 8
B_LOC = B // NCORES  # batches per core
SB = 128             # sequence block size
NTB = S // SB        # sequence blocks
BC = 16              # batches per matmul chunk -> N = BC*NB = 512 columns
NCHUNK_FULL = B_LOC // BC
F32 = np.float32

_PROGRAM_CACHE = {}


def _install_ntff_shim():
    """Provide antenv.axon_hooks if the image lacks it, so trace=True works.

    The axon boot module ships a ctypes NTFF-profile hook but only registers
    it when ``antenv.axon_hooks`` exists; this image's antenv does not have
    that module, which makes ``run_bass_kernel_spmd(trace=True)`` crash on
    import. Inject an in-memory equivalent. No-op if tracing is never used.
    """
    try:
        import antenv.axon_hooks  # noqa: F401
        return
    except ImportError:
        pass
    try:
        import types

        import antenv
        from trn_agent_boot.trn_boot import _ntff_profile_via_ctypes

        hook = _ntff_profile_via_ctypes("/opt/axon/libaxon_pjrt.so")
        mod = types.ModuleType("antenv.axon_hooks")
        state = {"hook": hook}
        mod.get_axon_ntff_profile_hook = lambda: state["hook"]
        mod.set_axon_ntff_profile_hook = lambda h: state.__setitem__("hook", h)
        sys.modules["antenv.axon_hooks"] = mod
        antenv.axon_hooks = mod
    except Exception:
        pass


def _split_multi_waits(nc, maxw=1):
    """Work around a walrus limit in this image: instructions carrying more
    than ~2 sem waits die in codegen with "Too many sync wait commands".
    Move excess waits onto same-engine NoOps placed just before the
    instruction (identical sync semantics, negligible cost)."""
    import concourse.mybir as mybir

    for fn in nc.m.functions:
        for blk in fn.blocks:
            out = []
            changed = False
            for inst in blk.instructions:
                si = inst.sync_info
                if si is not None and len(si.on_wait) > maxw:
                    waits = list(si.on_wait)
                    excess, keep = waits[:-maxw], waits[-maxw:]
                    for k, w in enumerate(excess):
                        out.append(mybir.InstNoOp(
                            name=f"{inst.name}-sw{k}",
                            engine=inst.engine,
                            bass_nofuse=True,
                            sync_info=mybir.SyncInfo(on_wait=[w], on_update=[]),
                        ))
                    inst.sync_info = mybir.SyncInfo(
                        on_wait=list(keep), on_update=list(si.on_update))
                    changed = True
                out.append(inst)
            if changed:
                blk.instructions = out
    return nc


def build_program(b_loc=B_LOC, split_waits=True):
    """Per-core Bass/Tile program. Device-side x/out layout is (S, b_loc, NB).

    x and out travel as bf16 (host converts): halves HBM traffic vs fp32,
    which is the binding roofline. Matmuls run bf16 (1 col/cycle @ 2.4 GHz,
    fp32 PSUM accumulate). The PSUM->SBUF bias-add+cast pass alternates
    between the DVE and Pool engines so neither becomes a serial tail.

    split_waits=True post-processes for the HW compiler; pass False when the
    module is destined for CoreSim (the sim rejects the injected NoOps)."""
    import concourse.bass as bass
    import concourse.mybir as mybir
    import concourse.tile as tile

    f32 = mybir.dt.float32
    bf16 = mybir.dt.bfloat16
    nchunk = b_loc // BC

    nc = bass.Bass("TRN2")
    x_h = nc.declare_dram_parameter("x", [S, b_loc, NB], bf16, False)
    # D^T and L^T packed side by side: one DMA with 512B-per-partition
    # descriptors (full rate) instead of two 256B-descriptor storms
    dl_h = nc.declare_dram_parameter("dlmat", [SB, 2 * SB], bf16, False)
    # pbias pre-transposed on host -> contiguous 1KB-per-partition DMA
    pb_h = nc.declare_dram_parameter("pbias", [SB, NTB, NB], f32, False)
    out_h = nc.declare_dram_parameter("out", [S, b_loc, NB], bf16, True)

    with tile.TileContext(nc) as tc:
        with (
            tc.tile_pool(name="consts", bufs=1) as cpool,
            tc.tile_pool(name="xin", bufs=6) as xpool,
            tc.tile_pool(name="outp", bufs=3) as opool,
            tc.tile_pool(name="tmp", bufs=6) as tpool,
            tc.tile_pool(name="psum", bufs=8, space="PSUM") as ppool,
        ):
            dl_sb = cpool.tile([SB, 2 * SB], bf16, tag="dl")
            pb_sb = cpool.tile([SB, NTB, NB], f32, tag="pb")
            # dl leads the sync queue (needed by the first matmul);
            # pbias leads the then-idle scalar queue so the first
            # consumer TTs aren't gated on it (on the slow gpsimd queue
            # it landed at ~17us and stalled the whole conveyor)
            nc.sync.dma_start(dl_sb[:], dl_h[:])
            nc.scalar.dma_start(pb_sb[:], pb_h[:])


            hb = b_loc // 2  # half-block batch split for finer DMA/sync
            # Bulk input stays on the sync queue alone, issued in-loop:
            # pre-issuing all blocks up front measurably SLOWS the queue
            # (274 -> ~180 GB/s, three attempts), and bulk DMAs issued
            # from scalar/gpsimd stall those engines' compute work
            # behind the issue (ring backpressure). SP issuing one block
            # per iteration with a deep-enough pool is the fast mode.
            prev_xt = None
            for tb in range(NTB):
                xt = xpool.tile([SB, b_loc, NB], bf16, tag="xt")
                r = slice(tb * SB, (tb + 1) * SB)
                if tb in (0, NTB - 1):
                    # split first and last blocks: the first so compute
                    # starts ~2us sooner, the last so its D-phase starts
                    # before the full block lands
                    nc.sync.dma_start(xt[:, :hb, :], x_h[r, :hb, :])
                    nc.sync.dma_start(xt[:, hb:, :], x_h[r, hb:, :])
                else:
                    nc.sync.dma_start(xt[:], x_h[r])
                ot = opool.tile([SB, b_loc, NB], bf16, tag="ot")
                bias = pb_sb[:, tb:tb + 1, :].broadcast_to((SB, BC, NB))
                # D phase then L phase (fewer stationary-weight switches);
                # consumers drain each half so its output DMA fires early
                for half in range(2):
                    cs = range(half * nchunk // 2, (half + 1) * nchunk // 2)
                    pss = {}
                    for c in cs:
                        bs = slice(c * BC, (c + 1) * BC)
                        ps = ppool.tile([SB, BC, NB], f32, tag="ps")
                        nc.tensor.matmul(ps[:], dl_sb[:, 0:SB], xt[:, bs, :],
                                         start=True, stop=(tb == 0))
                        pss[c] = ps
                    if tb > 0:
                        for c in cs:
                            bs = slice(c * BC, (c + 1) * BC)
                            nc.tensor.matmul(pss[c][:], dl_sb[:, SB:],
                                             prev_xt[:, bs, :],
                                             start=False, stop=True)
                    # PSUM -> SBUF bias-add + bf16 cast, split across
                    # engines: DVE handles most chunks directly (it can
                    # read PSUM); ACT copies the rest to a temp and GPSIMD
                    # (no PSUM access on TRN2) adds the bias from there.
                    # The slow ACT+GP path takes the half's FIRST chunks so
                    # a fast DVE chunk is what gates the output DMA.
                    # the last block's conveyor gates the kernel tail:
                    # keep slow GP off it entirely there
                    gp_chunks = () if tb == NTB - 1 else (0, 1, 4)
                    for c in cs:
                        bs = slice(c * BC, (c + 1) * BC)
                        if c in gp_chunks:
                            tmp = tpool.tile([SB, BC, NB], bf16, tag="tmp")
                            nc.scalar.copy(tmp[:], pss[c][:])
                            nc.gpsimd.tensor_tensor(ot[:, bs, :], tmp[:],
                                                    bias,
                                                    mybir.AluOpType.add)
                    for c in cs:
                        bs = slice(c * BC, (c + 1) * BC)
                        if c not in gp_chunks:
                            nc.vector.tensor_tensor(ot[:, bs, :], pss[c][:],
                                                    bias, mybir.AluOpType.add)
                    hs = slice(half * hb, (half + 1) * hb)
                    # outputs ride the scalar queue; for the last two
                    # blocks the h1 halves use sync's queue (idle once
                    # the input stream drains) so the final halves land
                    # in parallel
                    oeng = (nc.sync if (half == 1 and tb >= NTB - 2)
                            else nc.scalar)
                    oeng.dma_start(out_h[r, hs, :], ot[:, hs, :])
                prev_xt = xt
    return _split_multi_waits(nc) if split_waits else nc


def to_bf16(a):
    """Convert to bfloat16 (ml_dtypes) for the device-side bf16 datapath."""
    import ml_dtypes

    return np.ascontiguousarray(np.asarray(a, dtype=F32)).astype(
        ml_dtypes.bfloat16)


def host_consts(alpha, beta, pos_fwd_param, pos_bwd_param, past_steps):
    """Precompute D^T, L^T (128x128 FIR block matrices) and the position bias."""
    P = int(np.asarray(past_steps).reshape(-1)[0]) if np.ndim(past_steps) else int(past_steps)
    assert P <= SB, f"past_steps {P} > block size {SB} unsupported"
    a = float(np.asarray(alpha).reshape(-1)[0])
    b = float(np.asarray(beta).reshape(-1)[0])
    w = a * np.power(b, np.arange(P, dtype=np.float64))

    idx = np.arange(SB)
    km = idx[:, None] - idx[None, :]          # t - s
    D = np.where((km >= 1) & (km <= P), w[np.clip(km - 1, 0, P - 1)], 0.0)
    kml = km + SB                             # cross-block: t - s + 128
    L = np.where((kml >= 1) & (kml <= P), w[np.clip(kml - 1, 0, P - 1)], 0.0)
    DT = to_bf16(D.T)
    LT = to_bf16(L.T)

    t = np.arange(S)[:, None]
    j = np.arange(NB)[None, :]
    bucket = ((t - NB * j) % S) // NB         # (S, NB)
    pf = np.asarray(pos_fwd_param, dtype=np.float64).reshape(NB)
    pbw = np.asarray(pos_bwd_param, dtype=np.float64).reshape(NB)
    pb = pf[None, :] + pbw[bucket]            # (S, NB)
    pbias = np.ascontiguousarray(pb.reshape(NTB, SB, NB), dtype=F32)
    return DT, LT, pbias


def reference_numpy(x, alpha, beta, pos_fwd_param, pos_bwd_param, past_steps):
    """Float64 host reference (for self-tests)."""
    P = int(past_steps)
    a = float(np.asarray(alpha).reshape(-1)[0])
    b = float(np.asarray(beta).reshape(-1)[0])
    w = a * np.power(b, np.arange(P, dtype=np.float64))
    xf = np.asarray(x, dtype=np.float64)
    Bn, Sn, Dn = xf.shape
    y = np.zeros_like(xf)
    for i in range(P):
        y[:, i + 1:, :] += w[i] * xf[:, :Sn - 1 - i, :]
    t = np.arange(Sn)[:, None]
    j = np.arange(Dn)[None, :]
    bucket = ((t - Dn * j) % Sn) // Dn
    pf = np.asarray(pos_fwd_param, dtype=np.float64).reshape(Dn)
    pbw = np.asarray(pos_bwd_param, dtype=np.float64).reshape(Dn)
    return y + pf[None, :] + pbw[bucket]


def kernel(x, alpha, beta, pos_fwd_param, pos_bwd_param, past_steps):
    _install_ntff_shim()
    from concourse.bass_utils import run_bass_kernel_spmd

    x = np.asarray(x)
    assert x.shape == (B, S, NB), x.shape
    x = to_bf16(x)  # device datapath is bf16; halves HBM traffic
    DT, LT, pbias = host_consts(alpha, beta, pos_fwd_param, pos_bwd_param,
                                past_steps)

    if "hw" not in _PROGRAM_CACHE:
        _PROGRAM_CACHE["hw"] = build_program(B_LOC)
    nc = _PROGRAM_CACHE["hw"]

    core_ids = list(range(NCORES))
    DL = np.ascontiguousarray(np.concatenate([DT, LT], axis=1))
    pbias_t = np.ascontiguousarray(pbias.transpose(1, 0, 2))
    in_maps = [
        {
            # transposed view (S, B_LOC, NB); materialized by the runner's
            # input concat — no extra host copy vs contiguous sharding
            "x": x[i * B_LOC:(i + 1) * B_LOC].transpose(1, 0, 2),
            "dlmat": DL,
            "pbias": pbias_t,
        }
        for i in core_ids
    ]
    res = run_bass_kernel_spmd(nc, in_maps, core_ids)
    out = np.empty((B, S, NB), dtype=F32)
    for i in core_ids:
        out[i * B_LOC:(i + 1) * B_LOC] = (
            res.results[i]["out"].astype(F32).transpose(1, 0, 2))
    if res.exec_time_ns is not None:
        kernel.last_exec_time_ns = res.exec_time_ns
    kernel.last_results = res
    return out


kernel.last_exec_time_ns = None
kernel.last_results = None



# revision 34
# speedup vs baseline: 1.2147x; 1.0863x over previous
"""Trainium2 Bass kernel for nn_Attn_Pred_Model (causal geometric-decay FIR + position biases).

Math:
  out[b,t,d] = alpha * sum_{i=0}^{P-1} beta^i * x[b,t-1-i,d]
               + pos_fwd[d] + pos_bwd[bucket(t,d)]

The FIR along the sequence dim is a banded (block-bidiagonal) Toeplitz matmul:
with 128-row sequence blocks,  y[blk] = D @ x[blk] + L @ x[blk-1]
for two constant 128x128 matrices D, L built from (alpha, beta) on the host.
The (S, 32) position bias is precomputed on the host and added on the
vector engine after the PE matmuls.

Sharding: pure data parallelism — batch dim split across the 8 NeuronCores.
The device-side layout is (S, B_loc, NB): the shard handed to each core is a
transposed *view*; the SPMD runner's input-concat materializes it (same
one-copy cost as contiguous sharding) and in exchange every DMA descriptor
is a 2-16KB contiguous run instead of 128B, which is the difference between
~170 GB/s and ~line-rate HBM bandwidth per core.
"""

import os
import sys

import numpy as np

os.environ.setdefault("MYCRO_LOCAL_CACHE", "1")
if "/opt/trn_rl_repo" not in sys.path:
    sys.path.insert(0, "/opt/trn_rl_repo")

B, S, NB = 1024, 1024, 32
NCORES = 8
B_LOC = B // NCORES  # batches per core
SB = 128             # sequence block size
NTB = S // SB        # sequence blocks
BC = 16              # batches per matmul chunk -> N = BC*NB = 512 columns
NCHUNK_FULL = B_LOC // BC
F32 = np.float32

_PROGRAM_CACHE = {}


def _install_ntff_shim():
    """Provide antenv.axon_hooks if the image lacks it, so trace=True works.

    The axon boot module ships a ctypes NTFF-profile hook but only registers
    it when ``antenv.axon_hooks`` exists; this image's antenv does not have
    that module, which makes ``run_bass_kernel_spmd(trace=True)`` crash on
    import. Inject an in-memory equivalent. No-op if tracing is never used.
    """
    try:
        import antenv.axon_hooks  # noqa: F401
        return
    except ImportError:
        pass
    try:
        import types

        import antenv
        from trn_agent_boot.trn_boot import _ntff_profile_via_ctypes

        hook = _ntff_profile_via_ctypes("/opt/axon/libaxon_pjrt.so")
        mod = types.ModuleType("antenv.axon_hooks")
        state = {"hook": hook}
        mod.get_axon_ntff_profile_hook = lambda: state["hook"]
        mod.set_axon_ntff_profile_hook = lambda h: state.__setitem__("hook", h)
        sys.modules["antenv.axon_hooks"] = mod
        antenv.axon_hooks = mod
    except Exception:
        pass


def _split_multi_waits(nc, maxw=1):
    """Work around a walrus limit in this image: instructions carrying more
    than ~2 sem waits die in codegen with "Too many sync wait commands".
    Move excess waits onto same-engine NoOps placed just before the
    instruction (identical sync semantics, negligible cost)."""
    import concourse.mybir as mybir

    for fn in nc.m.functions:
        for blk in fn.blocks:
            out = []
            changed = False
            for inst in blk.instructions:
                si = inst.sync_info
                if si is not None and len(si.on_wait) > maxw:
                    waits = list(si.on_wait)
                    excess, keep = waits[:-maxw], waits[-maxw:]
                    for k, w in enumerate(excess):
                        out.append(mybir.InstNoOp(
                            name=f"{inst.name}-sw{k}",
                            engine=inst.engine,
                            bass_nofuse=True,
                            sync_info=mybir.SyncInfo(on_wait=[w], on_update=[]),
                        ))
                    inst.sync_info = mybir.SyncInfo(
                        on_wait=list(keep), on_update=list(si.on_update))
                    changed = True
                out.append(inst)
            if changed:
                blk.instructions = out
    return nc


def build_program(b_loc=B_LOC, split_waits=True):
    """Per-core Bass/Tile program. Device-side x/out layout is (S, b_loc, NB).

    x and out travel as bf16 (host converts): halves HBM traffic vs fp32,
    which is the binding roofline. Matmuls run bf16 (1 col/cycle @ 2.4 GHz,
    fp32 PSUM accumulate). The PSUM->SBUF bias-add+cast pass alternates
    between the DVE and Pool engines so neither becomes a serial tail.

    split_waits=True post-processes for the HW compiler; pass False when the
    module is destined for CoreSim (the sim rejects the injected NoOps)."""
    import concourse.bass as bass
    import concourse.mybir as mybir
    import concourse.tile as tile

    f32 = mybir.dt.float32
    bf16 = mybir.dt.bfloat16
    nchunk = b_loc // BC

    nc = bass.Bass("TRN2")
    x_h = nc.declare_dram_parameter("x", [S, b_loc, NB], bf16, False)
    # D^T and L^T packed side by side: one DMA with 512B-per-partition
    # descriptors (full rate) instead of two 256B-descriptor storms
    dl_h = nc.declare_dram_parameter("dlmat", [SB, 2 * SB], bf16, False)
    # pbias pre-transposed on host -> contiguous 1KB-per-partition DMA
    pb_h = nc.declare_dram_parameter("pbias", [SB, NTB, NB], f32, False)
    out_h = nc.declare_dram_parameter("out", [S, b_loc, NB], bf16, True)

    with tile.TileContext(nc) as tc:
        with (
            tc.tile_pool(name="consts", bufs=1) as cpool,
            tc.tile_pool(name="xin", bufs=6) as xpool,
            tc.tile_pool(name="outp", bufs=3) as opool,
            tc.tile_pool(name="tmp", bufs=6) as tpool,
            tc.tile_pool(name="psum", bufs=8, space="PSUM") as ppool,
        ):
            dl_sb = cpool.tile([SB, 2 * SB], bf16, tag="dl")
            pb_sb = cpool.tile([SB, NTB, NB], f32, tag="pb")
            # both const loads ride the then-idle scalar queue, keeping
            # the sync queue a pure x stream (const descriptors ahead of
            # x delay the first matmul; on the slow gpsimd queue pbias
            # landed at ~17us and stalled the whole conveyor)
            nc.scalar.dma_start(dl_sb[:], dl_h[:])
            nc.scalar.dma_start(pb_sb[:], pb_h[:])


            hb = b_loc // 2  # half-block batch split for finer DMA/sync

            # Bulk input stays on the sync queue alone, issued with a
            # one-block lookahead: pre-issuing everything up front
            # measurably SLOWS the queue (274 -> ~180 GB/s), bulk DMAs
            # from scalar/gpsimd stall those engines' compute behind the
            # issue, and in-loop issue without lookahead lets the
            # consumer-gated out(h1) DMAs head-of-line block the last
            # input blocks in the ring.
            xts = {}

            def issue_in(tb):
                xt = xpool.tile([SB, b_loc, NB], bf16, tag="xt",
                                name=f"xt{tb}")
                r_ = slice(tb * SB, (tb + 1) * SB)
                if tb in (0, NTB - 1):
                    # split first and last blocks: the first so compute
                    # starts ~2us sooner, the last so its D-phase starts
                    # before the full block lands
                    nc.sync.dma_start(xt[:, :hb, :], x_h[r_, :hb, :])
                    nc.sync.dma_start(xt[:, hb:, :], x_h[r_, hb:, :])
                else:
                    nc.sync.dma_start(xt[:], x_h[r_])
                xts[tb] = xt

            issue_in(0)
            issue_in(1)
            prev_xt = None
            for tb in range(NTB):
                if tb + 2 < NTB:
                    issue_in(tb + 2)
                xt = xts.pop(tb)
                r = slice(tb * SB, (tb + 1) * SB)
                ot = opool.tile([SB, b_loc, NB], bf16, tag="ot")
                bias = pb_sb[:, tb:tb + 1, :].broadcast_to((SB, BC, NB))
                # D phase then L phase (fewer stationary-weight switches);
                # consumers drain each half so its output DMA fires early
                for half in range(2):
                    cs = range(half * nchunk // 2, (half + 1) * nchunk // 2)
                    pss = {}
                    for c in cs:
                        bs = slice(c * BC, (c + 1) * BC)
                        ps = ppool.tile([SB, BC, NB], f32, tag="ps")
                        nc.tensor.matmul(ps[:], dl_sb[:, 0:SB], xt[:, bs, :],
                                         start=True, stop=(tb == 0))
                        pss[c] = ps
                    if tb > 0:
                        for c in cs:
                            bs = slice(c * BC, (c + 1) * BC)
                            nc.tensor.matmul(pss[c][:], dl_sb[:, SB:],
                                             prev_xt[:, bs, :],
                                             start=False, stop=True)
                    # PSUM -> SBUF bias-add + bf16 cast, split across
                    # engines: DVE handles most chunks directly (it can
                    # read PSUM); ACT copies the rest to a temp and GPSIMD
                    # (no PSUM access on TRN2) adds the bias from there.
                    # The slow ACT+GP path takes the half's FIRST chunks so
                    # a fast DVE chunk is what gates the output DMA.
                    # the last block's conveyor gates the kernel tail:
                    # keep slow GP off it entirely there
                    gp_chunks = () if tb == NTB - 1 else (0, 1, 4)
                    for c in cs:
                        bs = slice(c * BC, (c + 1) * BC)
                        if c in gp_chunks:
                            tmp = tpool.tile([SB, BC, NB], bf16, tag="tmp")
                            nc.scalar.copy(tmp[:], pss[c][:])
                            nc.gpsimd.tensor_tensor(ot[:, bs, :], tmp[:],
                                                    bias,
                                                    mybir.AluOpType.add)
                    for c in cs:
                        bs = slice(c * BC, (c + 1) * BC)
                        if c not in gp_chunks:
                            nc.vector.tensor_tensor(ot[:, bs, :], pss[c][:],
                                                    bias, mybir.AluOpType.add)
                    hs = slice(half * hb, (half + 1) * hb)
                    # outputs ride the scalar queue; for the last two
                    # blocks the h1 halves use sync's queue (idle once
                    # the input stream drains) so the final halves land
                    # in parallel
                    oeng = (nc.sync if (half == 1 and tb >= NTB - 2)
                            else nc.scalar)
                    oeng.dma_start(out_h[r, hs, :], ot[:, hs, :])
                prev_xt = xt
    return _split_multi_waits(nc) if split_waits else nc


def to_bf16(a):
    """Convert to bfloat16 (ml_dtypes) for the device-side bf16 datapath."""
    import ml_dtypes

    return np.ascontiguousarray(np.asarray(a, dtype=F32)).astype(
        ml_dtypes.bfloat16)


def host_consts(alpha, beta, pos_fwd_param, pos_bwd_param, past_steps):
    """Precompute D^T, L^T (128x128 FIR block matrices) and the position bias."""
    P = int(np.asarray(past_steps).reshape(-1)[0]) if np.ndim(past_steps) else int(past_steps)
    assert P <= SB, f"past_steps {P} > block size {SB} unsupported"
    a = float(np.asarray(alpha).reshape(-1)[0])
    b = float(np.asarray(beta).reshape(-1)[0])
    w = a * np.power(b, np.arange(P, dtype=np.float64))

    idx = np.arange(SB)
    km = idx[:, None] - idx[None, :]          # t - s
    D = np.where((km >= 1) & (km <= P), w[np.clip(km - 1, 0, P - 1)], 0.0)
    kml = km + SB                             # cross-block: t - s + 128
    L = np.where((kml >= 1) & (kml <= P), w[np.clip(kml - 1, 0, P - 1)], 0.0)
    DT = to_bf16(D.T)
    LT = to_bf16(L.T)

    t = np.arange(S)[:, None]
    j = np.arange(NB)[None, :]
    bucket = ((t - NB * j) % S) // NB         # (S, NB)
    pf = np.asarray(pos_fwd_param, dtype=np.float64).reshape(NB)
    pbw = np.asarray(pos_bwd_param, dtype=np.float64).reshape(NB)
    pb = pf[None, :] + pbw[bucket]            # (S, NB)
    pbias = np.ascontiguousarray(pb.reshape(NTB, SB, NB), dtype=F32)
    return DT, LT, pbias


def reference_numpy(x, alpha, beta, pos_fwd_param, pos_bwd_param, past_steps):
    """Float64 host reference (for self-tests)."""
    P = int(past_steps)
    a = float(np.asarray(alpha).reshape(-1)[0])
    b = float(np.asarray(beta).reshape(-1)[0])
    w = a * np.power(b, np.arange(P, dtype=np.float64))
    xf = np.asarray(x, dtype=np.float64)
    Bn, Sn, Dn = xf.shape
    y = np.zeros_like(xf)
    for i in range(P):
        y[:, i + 1:, :] += w[i] * xf[:, :Sn - 1 - i, :]
    t = np.arange(Sn)[:, None]
    j = np.arange(Dn)[None, :]
    bucket = ((t - Dn * j) % Sn) // Dn
    pf = np.asarray(pos_fwd_param, dtype=np.float64).reshape(Dn)
    pbw = np.asarray(pos_bwd_param, dtype=np.float64).reshape(Dn)
    return y + pf[None, :] + pbw[bucket]


def kernel(x, alpha, beta, pos_fwd_param, pos_bwd_param, past_steps):
    _install_ntff_shim()
    from concourse.bass_utils import run_bass_kernel_spmd

    x = np.asarray(x)
    assert x.shape == (B, S, NB), x.shape
    x = to_bf16(x)  # device datapath is bf16; halves HBM traffic
    DT, LT, pbias = host_consts(alpha, beta, pos_fwd_param, pos_bwd_param,
                                past_steps)

    if "hw" not in _PROGRAM_CACHE:
        _PROGRAM_CACHE["hw"] = build_program(B_LOC)
    nc = _PROGRAM_CACHE["hw"]

    core_ids = list(range(NCORES))
    DL = np.ascontiguousarray(np.concatenate([DT, LT], axis=1))
    pbias_t = np.ascontiguousarray(pbias.transpose(1, 0, 2))
    in_maps = [
        {
            # transposed view (S, B_LOC, NB); materialized by the runner's
            # input concat — no extra host copy vs contiguous sharding
            "x": x[i * B_LOC:(i + 1) * B_LOC].transpose(1, 0, 2),
            "dlmat": DL,
            "pbias": pbias_t,
        }
        for i in core_ids
    ]
    res = run_bass_kernel_spmd(nc, in_maps, core_ids)
    out = np.empty((B, S, NB), dtype=F32)
    for i in core_ids:
        out[i * B_LOC:(i + 1) * B_LOC] = (
            res.results[i]["out"].astype(F32).transpose(1, 0, 2))
    if res.exec_time_ns is not None:
        kernel.last_exec_time_ns = res.exec_time_ns
    kernel.last_results = res
    return out


kernel.last_exec_time_ns = None
kernel.last_results = None

